# revision 6
# baseline (speedup 1.0000x reference)
"""Trainium2 Bass kernel for nms_detection bandwidth-budgeted BEV selection.

Contract: kernel(**inputs) takes FULL inputs
  - collab_bev_data_list [4, 90, 192, 192] f32
  - utility_map_list     [4, 192, 192, 3]  f32
  - bandwidth_budget     scalar
and returns (sparse [4,90,192,192] f32, sel_idx [4,192,192] f32), matching
the reference bit-exactly.

Sharding: data-parallel over samples; 8 cores = 4 samples x 2 channel-halves.
Each core computes its sample's greedy-knapsack selection redundantly (cheap)
and builds the masked BEV for its 45-channel half. Channels are permuted on
the host so each core sees fixed group segments [0:5]=vox(g0), [5:37]=feat(g1),
[37:45]=det(g2), keeping the SPMD program identical across cores.

Device algorithm (per sample, N=36864 pixels as [128 partitions x 288]):
  best_u/argmax/cost per pixel; greedy budget scan over utility-descending
  order == accept all valid pixels with u > v0 (v0 found by binary search on
  the cost-weighted tail sum; all sums are small integers, exact in f32),
  plus an exact <=4-acceptance boundary walk over per-class top-8 candidates.
"""

import numpy as np

import concourse.bacc as bacc
import concourse.bass as bass
import concourse.mybir as mybir
import concourse.tile as tile
from concourse import bass_isa
from concourse.bass_utils import run_bass_kernel_spmd

F32 = mybir.dt.float32
U8 = mybir.dt.uint8
ALU = mybir.AluOpType
AX = mybir.AxisListType
ROP = bass_isa.ReduceOp

P = 128            # SBUF partitions
J = 288            # pixels per partition (P*J == 36864 == 192*192)
N_PIX = P * J
NCH = 45           # channels per core (half of 90)
CH_CHUNK = 15      # channels per DMA/compute chunk
N_ROUNDS = 15      # 3-probe search rounds (4^15 interval shrink, sub-ULP)
NEG = -1.0e30
POS = 1.0e30

# group segments within each core's (permuted) 45 channels: [lo, hi, group]
SEGMENTS = [(0, 5, 0.0), (5, 37, 1.0), (37, 45, 2.0)]

# host-side channel permutation: half h gets vox[5h:5h+5], feat[32h:32h+32],
# det[8h:8h+8] (global channel ids; vox=0..9, feat=10..73, det=74..89)
def _half_perm(h):
    return (list(range(5 * h, 5 * h + 5))
            + list(range(10 + 32 * h, 10 + 32 * h + 32))
            + list(range(74 + 8 * h, 74 + 8 * h + 8)))


def _build_nc():
    nc = bacc.Bacc(None, target_bir_lowering=False, debug=False)

    util = nc.declare_dram_parameter("util", [P, J * 3], F32, isOutput=False)
    bud = nc.declare_dram_parameter("bud", [P, 1], F32, isOutput=False)
    bev = nc.declare_dram_parameter("bev", [NCH, N_PIX], F32, isOutput=False)
    sel_o = nc.declare_dram_parameter("sel_o", [P, J], F32, isOutput=True)
    bev_o = nc.declare_dram_parameter("bev_o", [NCH, N_PIX], F32, isOutput=True)

    with tile.TileContext(nc) as tc:
        with (
            tc.tile_pool(name="big", bufs=1) as bigp,
            tc.tile_pool(name="st", bufs=1) as stp,
            tc.tile_pool(name="bevp", bufs=3) as bevp,
        ):
            def big(tag):
                return bigp.tile([P, J], F32, name=tag, tag=tag)

            def st(tag, free=1):
                return stp.tile([P, free], F32, name=tag, tag=tag)

            def stu8(tag, free=1):
                return stp.tile([P, free], U8, name=tag, tag=tag)

            def bigu8(tag):
                return bigp.tile([P, J], U8, name=tag, tag=tag)

            # ---------------- load utility + budget ----------------
            ut = bigp.tile([P, J * 3], F32, name="ut", tag="ut")
            nc.sync.dma_start(out=ut[:], in_=util[:])
            budt = st("budt")
            nc.sync.dma_start(out=budt[:], in_=bud[:])

            ut3 = ut[:].rearrange("p (j g) -> p j g", g=3)

            # ---------------- per-pixel prep ----------------
            bu = big("bu")
            nc.vector.tensor_reduce(out=bu[:], in_=ut3, axis=AX.X, op=ALU.max)
            bu3 = bu[:].rearrange("p (j o) -> p j o", o=1)

            valid = big("valid")
            nc.vector.tensor_single_scalar(out=valid[:], in_=bu[:], scalar=0.0,
                                           op=ALU.is_gt)
            # argmax-first flags
            e0 = big("e0")
            nc.vector.tensor_tensor(out=e0[:].rearrange("p (j o) -> p j o", o=1),
                                    in0=ut3[:, :, 0:1], in1=bu3, op=ALU.is_ge)
            e1 = big("e1")
            nc.vector.tensor_tensor(out=e1[:].rearrange("p (j o) -> p j o", o=1),
                                    in0=ut3[:, :, 1:2], in1=bu3, op=ALU.is_ge)
            ne0 = big("ne0")
            nc.vector.tensor_single_scalar(out=ne0[:], in_=e0[:], scalar=0.0,
                                           op=ALU.is_equal)
            nc.vector.tensor_tensor(out=e1[:], in0=e1[:], in1=ne0[:], op=ALU.mult)
            ne1 = big("ne1")
            nc.vector.tensor_single_scalar(out=ne1[:], in_=e1[:], scalar=0.0,
                                           op=ALU.is_equal)
            e2 = big("e2")
            nc.vector.tensor_tensor(out=e2[:], in0=ne0[:], in1=ne1[:], op=ALU.mult)

            # gmap = e1 + 2*e2 ; cost = 10*e0 + 5*e1 + 2*e2 ; wcost = cost*valid
            gmap = big("gmap")
            nc.vector.scalar_tensor_tensor(out=gmap[:], in0=e2[:], scalar=2.0,
                                           in1=e1[:], op0=ALU.mult, op1=ALU.add)
            tmp = big("tmp")
            nc.vector.scalar_tensor_tensor(out=tmp[:], in0=e2[:], scalar=2.0,
                                           in1=e1[:], op0=ALU.mult, op1=ALU.bypass)
            # tmp = 2*e2 ; tmp = 5*e1 + tmp ; wcost = (10*e0 + tmp) * valid
            nc.vector.scalar_tensor_tensor(out=tmp[:], in0=e1[:], scalar=5.0,
                                           in1=tmp[:], op0=ALU.mult, op1=ALU.add)
            wcost = big("wcost")
            nc.vector.scalar_tensor_tensor(out=wcost[:], in0=e0[:], scalar=10.0,
                                           in1=tmp[:], op0=ALU.mult, op1=ALU.add)
            nc.vector.tensor_tensor(out=wcost[:], in0=wcost[:], in1=valid[:],
                                    op=ALU.mult)

            # total + global max
            par = st("par")
            nc.vector.tensor_reduce(out=par[:], in_=wcost[:], axis=AX.X, op=ALU.add)
            totr = st("totr")
            nc.gpsimd.partition_all_reduce(totr[:], par[:], channels=P,
                                           reduce_op=ROP.add)
            pmax = st("pmax")
            nc.vector.tensor_reduce(out=pmax[:], in_=bu[:], axis=AX.X, op=ALU.max)
            gmaxr = st("gmaxr")
            nc.gpsimd.partition_all_reduce(gmaxr[:], pmax[:], channels=P,
                                           reduce_op=ROP.max)

            # ---------------- binary search for v0 ----------------
            junk = big("junk")
            mids = st("mids", 3)
            t3 = st("t3", 3)
            par3 = st("par3", 3)
            lomc = st("lomc", 3)
            hia = st("hia", 3)
            hib = st("hib", 3)
            lom = st("lom")
            him = st("him")
            d = st("d")

            lo = [st("lo_a"), st("lo_b")]
            hi = [st("hi_a"), st("hi_b")]
            nc.vector.memset(lo[0][:], 0.0)
            nc.vector.tensor_copy(out=hi[0][:], in_=gmaxr[:])

            for it in range(N_ROUNDS):
                cl, nl = lo[it % 2], lo[(it + 1) % 2]
                ch, nh = hi[it % 2], hi[(it + 1) % 2]
                # d = (hi - lo) * 0.25 ; mids = lo+d, lo+2d, lo+3d
                nc.vector.tensor_scalar(out=d[:], in0=ch[:], scalar1=cl[:],
                                        scalar2=0.25, op0=ALU.subtract,
                                        op1=ALU.mult)
                nc.vector.tensor_single_scalar(out=mids[:, 0:1], in_=cl[:],
                                               scalar=d[:], op=ALU.add)
                nc.vector.tensor_single_scalar(out=mids[:, 1:2], in_=mids[:, 0:1],
                                               scalar=d[:], op=ALU.add)
                nc.vector.tensor_single_scalar(out=mids[:, 2:3], in_=mids[:, 1:2],
                                               scalar=d[:], op=ALU.add)
                for k in range(3):
                    nc.vector.scalar_tensor_tensor(
                        out=junk[:], in0=bu[:], scalar=mids[:, k:k + 1],
                        in1=wcost[:], op0=ALU.is_gt, op1=ALU.mult,
                        accum_out=par3[:, k:k + 1])
                nc.gpsimd.partition_all_reduce(t3[:], par3[:], channels=P,
                                               reduce_op=ROP.add)
                # lo' = max(lo, max_k mids_k where T_k > B)   (mids > 0 always)
                nc.vector.scalar_tensor_tensor(out=lomc[:], in0=t3[:],
                                               scalar=budt[:], in1=mids[:],
                                               op0=ALU.is_gt, op1=ALU.mult)
                nc.vector.tensor_reduce(out=lom[:], in_=lomc[:], axis=AX.X,
                                        op=ALU.max)
                nc.vector.tensor_single_scalar(out=nl[:], in_=lom[:],
                                               scalar=cl[:], op=ALU.max)
                # hi' = min(hi, min_k mids_k where T_k <= B)
                nc.vector.scalar_tensor_tensor(out=hia[:], in0=t3[:],
                                               scalar=budt[:], in1=mids[:],
                                               op0=ALU.is_le, op1=ALU.mult)
                nc.vector.tensor_scalar(out=hib[:], in0=t3[:], scalar1=budt[:],
                                        scalar2=POS, op0=ALU.is_gt, op1=ALU.mult)
                nc.vector.tensor_tensor(out=hia[:], in0=hia[:], in1=hib[:],
                                        op=ALU.add)
                nc.vector.tensor_reduce(out=him[:], in_=hia[:], axis=AX.X,
                                        op=ALU.min)
                nc.vector.tensor_single_scalar(out=nh[:], in_=him[:],
                                               scalar=ch[:], op=ALU.min)

            flo = lo[N_ROUNDS % 2]
            fhi = hi[N_ROUNDS % 2]

            # ---------------- v0, usage, bypass ----------------
            # v0 = global max of (bu <= hi) * bu   (invalid bu<=0 can't win)
            nc.vector.scalar_tensor_tensor(out=junk[:], in0=bu[:], scalar=fhi[:],
                                           in1=bu[:], op0=ALU.is_le, op1=ALU.mult)
            nc.vector.tensor_reduce(out=par[:], in_=junk[:], axis=AX.X, op=ALU.max)
            v0r = st("v0r")
            nc.gpsimd.partition_all_reduce(v0r[:], par[:], channels=P,
                                           reduce_op=ROP.max)
            negt = st("negt")
            nc.vector.memset(negt[:], NEG)
            predt = stu8("predt")
            nc.vector.tensor_single_scalar(out=predt[:], in_=totr[:],
                                           scalar=budt[:], op=ALU.is_le)
            nc.vector.copy_predicated(out=v0r[:], mask=predt[:], data=negt[:])

            # usage = sum (bu > v0)*wcost ; r = bud - usage (0 if bypass)
            nc.vector.scalar_tensor_tensor(out=junk[:], in0=bu[:], scalar=v0r[:],
                                           in1=wcost[:], op0=ALU.is_gt,
                                           op1=ALU.mult, accum_out=par[:])
            usager = st("usager")
            nc.gpsimd.partition_all_reduce(usager[:], par[:], channels=P,
                                           reduce_op=ROP.add)
            rr = st("rr")
            nc.vector.tensor_scalar(out=rr[:], in0=usager[:], scalar1=budt[:],
                                    scalar2=-1.0, op0=ALU.subtract, op1=ALU.mult)
            zt = st("zt")
            nc.vector.memset(zt[:], 0.0)
            nc.vector.copy_predicated(out=rr[:], mask=predt[:], data=zt[:])

            # ---------------- phase-1 mask + per-class arrays ----------------
            selm = bigu8("selm")
            nc.vector.scalar_tensor_tensor(out=selm[:], in0=bu[:], scalar=v0r[:],
                                           in1=valid[:], op0=ALU.is_gt,
                                           op1=ALU.mult)

            top24 = stp.tile([P, 24], F32, name="top24", tag="top24")
            mvs = []
            for c in range(3):
                clsm = bigu8(f"clsm{c}")
                nc.vector.tensor_single_scalar(out=clsm[:], in_=gmap[:],
                                               scalar=float(c), op=ALU.is_equal)
                nc.vector.scalar_tensor_tensor(out=clsm[:], in0=bu[:],
                                               scalar=v0r[:], in1=clsm[:],
                                               op0=ALU.is_le, op1=ALU.mult)
                nc.vector.tensor_tensor(out=clsm[:], in0=clsm[:], in1=valid[:],
                                        op=ALU.mult)
                mv = big(f"mv{c}")
                nc.vector.memset(mv[:], NEG)
                nc.vector.copy_predicated(out=mv[:], mask=clsm[:], data=bu[:])
                nc.vector.max(out=top24[:, 8 * c:8 * (c + 1)], in_=mv[:])
                mvs.append(mv)

            # ---------------- phase-2 walk (<=4 acceptances) ----------------
            costs3 = st("costs3", 3)
            nc.vector.memset(costs3[:, 0:1], 10.0)
            nc.vector.memset(costs3[:, 1:2], 5.0)
            nc.vector.memset(costs3[:, 2:3], 2.0)
            cur3 = st("cur3", 3)
            nc.vector.memset(cur3[:], POS)
            neg3 = st("neg3", 3)
            nc.vector.memset(neg3[:], NEG)

            t24 = stp.tile([P, 24], F32, name="t24", tag="t24")
            hm3 = st("hm3", 3)
            gm3 = st("gm3", 3)
            fit3 = stu8("fit3", 3)
            pos3 = stu8("pos3", 3)
            hf3 = st("hf3", 3)
            p3 = stu8("p3", 3)
            pa3 = stu8("pa3", 3)
            best = st("best")
            anyp = stu8("anyp")
            np0 = stu8("np0")
            np1 = stu8("np1")
            npb = stu8("npb")
            dd3 = st("dd3", 3)
            dec = st("dec")

            top3v = top24[:].rearrange("p (c k) -> p c k", c=3)
            t24v = t24[:].rearrange("p (c k) -> p c k", c=3)
            curb = cur3[:].rearrange("p (c o) -> p c o", o=1).broadcast_to([P, 3, 8])

            for step in range(4):
                # masked candidates below cur per class; -1e30 entries -> 0 (ok)
                nc.vector.tensor_tensor(out=t24v, in0=top3v, in1=curb, op=ALU.is_lt)
                nc.vector.tensor_tensor(out=t24[:], in0=t24[:], in1=top24[:],
                                        op=ALU.mult)
                nc.vector.tensor_reduce(out=hm3[:], in_=t24v, axis=AX.X, op=ALU.max)
                nc.gpsimd.partition_all_reduce(gm3[:], hm3[:], channels=P,
                                               reduce_op=ROP.max)
                nc.vector.tensor_single_scalar(out=fit3[:], in_=costs3[:],
                                               scalar=rr[:], op=ALU.is_le)
                nc.vector.tensor_single_scalar(out=pos3[:], in_=gm3[:], scalar=0.0,
                                               op=ALU.is_gt)
                nc.vector.tensor_tensor(out=fit3[:], in0=fit3[:], in1=pos3[:],
                                        op=ALU.mult)
                nc.vector.tensor_copy(out=hf3[:], in_=neg3[:])
                nc.vector.copy_predicated(out=hf3[:], mask=fit3[:], data=gm3[:])
                nc.vector.tensor_reduce(out=best[:], in_=hf3[:], axis=AX.X,
                                        op=ALU.max)
                nc.vector.tensor_single_scalar(out=anyp[:], in_=best[:],
                                               scalar=-1.0e29, op=ALU.is_gt)
                nc.vector.tensor_single_scalar(out=p3[:], in_=hf3[:],
                                               scalar=best[:], op=ALU.is_ge)
                # first-true priority + gate by anyp
                nc.vector.tensor_single_scalar(out=np0[:], in_=p3[:, 0:1],
                                               scalar=0.0, op=ALU.is_equal)
                nc.vector.tensor_tensor(out=pa3[:, 0:1], in0=p3[:, 0:1],
                                        in1=anyp[:], op=ALU.mult)
                nc.vector.tensor_tensor(out=p3[:, 1:2], in0=p3[:, 1:2],
                                        in1=np0[:], op=ALU.mult)
                nc.vector.tensor_single_scalar(out=np1[:], in_=p3[:, 1:2],
                                               scalar=0.0, op=ALU.is_equal)
                nc.vector.tensor_tensor(out=pa3[:, 1:2], in0=p3[:, 1:2],
                                        in1=anyp[:], op=ALU.mult)
                nc.vector.tensor_tensor(out=npb[:], in0=np0[:], in1=np1[:],
                                        op=ALU.mult)
                nc.vector.tensor_tensor(out=p3[:, 2:3], in0=p3[:, 2:3],
                                        in1=npb[:], op=ALU.mult)
                nc.vector.tensor_tensor(out=pa3[:, 2:3], in0=p3[:, 2:3],
                                        in1=anyp[:], op=ALU.mult)
                # r -= chosen cost
                nc.vector.tensor_tensor(out=dd3[:], in0=pa3[:], in1=costs3[:],
                                        op=ALU.mult)
                nc.vector.tensor_reduce(out=dec[:], in_=dd3[:], axis=AX.X,
                                        op=ALU.add)
                nc.vector.tensor_tensor(out=rr[:], in0=rr[:], in1=dec[:],
                                        op=ALU.subtract)
                # cur3 = where(pa3, best, cur3)
                nc.vector.copy_predicated(out=cur3[:], mask=pa3[:],
                                          data=best[:].broadcast_to([P, 3]))

            # ---------------- final selection map ----------------
            mk = bigu8("mk")
            for c in range(3):
                nc.vector.tensor_single_scalar(out=mk[:], in_=mvs[c][:],
                                               scalar=cur3[:, c:c + 1],
                                               op=ALU.is_ge)
                nc.vector.tensor_tensor(out=selm[:], in0=selm[:], in1=mk[:],
                                        op=ALU.max)
            si = big("si")
            nc.vector.memset(si[:], -1.0)
            nc.vector.copy_predicated(out=si[:], mask=selm[:], data=gmap[:])
            nc.sync.dma_start(out=sel_o[:], in_=si[:])

            # ---------------- BEV mask + move ----------------
            gconst = {}
            for (_s0, _s1, g) in SEGMENTS:
                gt = st(f"g{int(g)}")
                nc.vector.memset(gt[:], g)
                gconst[g] = gt
            n_chunks = NCH // CH_CHUNK
            for k in range(n_chunks):
                c0, c1 = k * CH_CHUNK, (k + 1) * CH_CHUNK
                data = bevp.tile([P, CH_CHUNK * J], F32, name="bevc", tag="bevc")
                src = bev[c0:c1, :].rearrange("c (p j) -> p c j", p=P)
                nc.sync.dma_start(out=data[:].rearrange("p (c j) -> p c j",
                                                        c=CH_CHUNK), in_=src)
                for (s0, s1, g) in SEGMENTS:
                    a, b = max(s0, c0), min(s1, c1)
                    if a >= b:
                        continue
                    nch = b - a
                    dv = data[:, (a - c0) * J:(b - c0) * J]
                    dv3 = dv.rearrange("p (c j) -> p c j", c=nch)
                    sib = si[:].rearrange("p (o j) -> p o j", o=1) \
                               .broadcast_to([P, nch, J])
                    gi = gconst[g]
                    nc.vector.scalar_tensor_tensor(out=dv3, in0=sib,
                                                   scalar=gi[:], in1=dv3,
                                                   op0=ALU.is_equal,
                                                   op1=ALU.mult)
                dst = bev_o[c0:c1, :].rearrange("c (p j) -> p c j", p=P)
                nc.sync.dma_start(out=dst,
                                  in_=data[:].rearrange("p (c j) -> p c j",
                                                        c=CH_CHUNK))

    nc.compile()
    return nc


_NC_CACHE = {}


def _get_nc():
    if "nc" not in _NC_CACHE:
        _NC_CACHE["nc"] = _build_nc()
    return _NC_CACHE["nc"]


def _run(collab_bev_data_list, utility_map_list, bandwidth_budget, **spmd_kwargs):
    collab = np.ascontiguousarray(np.asarray(collab_bev_data_list, np.float32))
    utility = np.ascontiguousarray(np.asarray(utility_map_list, np.float32))
    Bn, C, H, W = collab.shape
    budget = np.float32(float(bandwidth_budget) / Bn)

    perms = [_half_perm(0), _half_perm(1)]
    in_maps = []
    for core in range(8):
        s, h = core // 2, core % 2
        in_maps.append({
            "util": utility[s].reshape(P, J * 3),
            "bud": np.full((P, 1), budget, np.float32),
            "bev": collab[s][perms[h]].reshape(NCH, N_PIX),
        })

    nc = _get_nc()
    res = run_bass_kernel_spmd(nc, in_maps, core_ids=list(range(8)),
                               **spmd_kwargs)
    results = res.results

    sparse = np.empty((Bn, C, H, W), np.float32)
    sel_idx = np.empty((Bn, H, W), np.float32)
    for core in range(8):
        s, h = core // 2, core % 2
        out_bev = results[core]["bev_o"].reshape(NCH, H, W)
        sparse[s, perms[h]] = out_bev
        if h == 0:
            sel_idx[s] = results[core]["sel_o"].reshape(H, W)
    return sparse, sel_idx, res


def kernel(collab_bev_data_list, utility_map_list, bandwidth_budget):
    sparse, sel_idx, _ = _run(collab_bev_data_list, utility_map_list,
                              bandwidth_budget)
    return sparse, sel_idx
